# revision 5
# baseline (speedup 1.0000x reference)
"""Trainium2 Bass kernel for nn_AtomicHAR.

Strategy
--------
* The top-20% cutoff + per-batch segmentation in the reference is decided on
  exact f32 loss values that contain bit-level ties at the cutoff, so the
  decision path (CNN -> bridge -> forecast loss) is replayed with the exact
  same eager jax op sequence as the reference (bit-identical on this backend).
  Those bit-exact tensors also serve as the forecast-family outputs.
* The heavy generated outputs (imu decoder and the atom encoder/decoder,
  including the [A, 6, 1280] atom_gen) run in a single Bass NEFF, SPMD over
  8 NeuronCores: imu decoder data-parallel over batch, atom network
  data-parallel over atoms.
* imu_atoms / imu_atoms_mask are pure gather/pad rearrangements of the raw
  input x, assembled host-side (exact).
"""

import sys
import types

import numpy as np

MAL = 10
BS, SEQ, C, L = 128, 128, 6, 20
N = BS * SEQ
D, H = 64, 256
NCORES = 8
BS_LOC = BS // NCORES          # batches per core
NW = BS_LOC * SEQ              # windows per core
WTOT = MAL * SEQ               # 1280
QW = WTOT // L                 # 64 source windows per atom row

LAST_RESULTS = []              # BassKernelResults of the bass launch(es)


# ----------------------------------------------------------------------------
# NTFF trace hook shim: make `trace=True` (or BASS_TRACE=1) safe even when the
# image's antenv package lacks axon_hooks.
# ----------------------------------------------------------------------------
def _ensure_ntff_hook():
    try:
        import antenv.axon_hooks  # noqa: F401
        return
    except Exception:
        pass
    try:
        import antenv
    except Exception:
        return
    mod = types.ModuleType('antenv.axon_hooks')
    state = {'hook': None}
    mod.set_axon_ntff_profile_hook = lambda h: state.__setitem__('hook', h)
    mod.get_axon_ntff_profile_hook = lambda: state['hook']
    sys.modules['antenv.axon_hooks'] = mod
    antenv.axon_hooks = mod
    try:
        from trn_agent_boot.trn_boot import _ntff_profile_via_ctypes
        hook = _ntff_profile_via_ctypes('/opt/axon/libaxon_pjrt.so')
        if hook is not None:
            mod.set_axon_ntff_profile_hook(hook)
    except Exception:
        pass


# ----------------------------------------------------------------------------
# Phase 1 replay: the exact op sequence of the reference through forcast_loss.
# ----------------------------------------------------------------------------
def _replay_phase1(x, imu_mask, c1_w, c1_b, c2_w, c2_b, c3_w, c3_b,
                   br_w, br_b, f1_w, f1_b, f2_w, f2_b):
    import jax
    import jax.numpy as jnp
    from jax import lax

    def _conv1d(x_, w, b):
        pad = (w.shape[-1] - 1) // 2
        y = lax.conv_general_dilated(x_, w, (1,), [(pad, pad)],
                                     dimension_numbers=('NCH', 'OIH', 'NCH'))
        return y + b[None, :, None]

    def _maxpool2(x_):
        return lax.reduce_window(x_, -jnp.inf, lax.max, (1, 1, 2), (1, 1, 2),
                                 'VALID')

    bs, seq, Cc, Ll = x.shape
    n = bs * seq
    h = _maxpool2(jax.nn.relu(_conv1d(x.reshape(n, Cc, Ll), c1_w, c1_b)))
    h = _maxpool2(jax.nn.relu(_conv1d(h, c2_w, c2_b)))
    h = _maxpool2(jax.nn.relu(_conv1d(h, c3_w, c3_b)))
    cnn_out = h.reshape(n, -1)
    bridge_out = jax.nn.sigmoid(cnn_out @ br_w + br_b)
    d = bridge_out.shape[1]
    forcast_in3 = bridge_out.reshape(bs, seq, d)
    shft = jnp.pad(forcast_in3, ((0, 0), (1, 0), (0, 0)))[:, :-1]
    fmask = jnp.ones((bs, seq, d), jnp.float32).at[:, 0, :].set(0.0).reshape(n, d)
    fmask = fmask * imu_mask[:, :, 0, 0].reshape(n, 1)
    forcast_in = forcast_in3.reshape(n, d)
    feat = jax.nn.relu(shft.reshape(n, d) @ f1_w + f1_b)
    forcast = feat @ f2_w + f2_b
    forcast_loss = jnp.mean(jnp.square((forcast - forcast_in) * fmask), axis=1)
    return forcast_in3, forcast_in, forcast, fmask, forcast_loss


def _segment(l2, cutoff):
    """Exact mirror of the reference's host segmentation loop."""
    seg_lists = []
    for b in range(l2.shape[0]):
        idxs = np.nonzero(l2[b] > cutoff)[0]
        segs = []
        if idxs.shape[0] > 0:
            for i in range(idxs.shape[0] - 1):
                if int(idxs[i + 1] - idxs[i]) > MAL:
                    cur = int(idxs[i]); segs.append(cur)
                    while cur < int(idxs[i + 1]):
                        cur += MAL
                        if cur >= int(idxs[i + 1]):
                            break
                        segs.append(cur)
                else:
                    segs.append(int(idxs[i]))
            segs.append(int(idxs[-1]))
        seg_lists.append(segs)
    return seg_lists


# ----------------------------------------------------------------------------
# Dense per-window conv matrices (conv as matmul over the flattened window).
# ----------------------------------------------------------------------------
def _conv_W(w, L_in):
    """[I*L_in, O*L_in] dense matrix of a 'same' K-tap conv1d.
    Row (i, li) = i*L_in+li, col (o, lo) = o*L_in+lo."""
    O, I, K = w.shape
    pad = (K - 1) // 2
    W = np.zeros((I * L_in, O * L_in), np.float32)
    lo = np.arange(L_in)
    for o in range(O):
        for i in range(I):
            for k in range(K):
                li = lo + k - pad
                m = (li >= 0) & (li < L_in)
                W[i * L_in + li[m], o * L_in + lo[m]] += w[o, i, k]
    return W


def _bf16(a):
    import ml_dtypes
    return np.ascontiguousarray(np.asarray(a, np.float32)).astype(ml_dtypes.bfloat16)


# ----------------------------------------------------------------------------
# The Bass kernel: one NEFF, SPMD across 8 cores.
#   imu decoder: bridgeT [64, NW] -> imu [NW, 120]
#   atom net:    afT chunks [128, 5, A_pad] -> atom [A_pad, 7680]
# ----------------------------------------------------------------------------
_KERNEL_CACHE = {}


def _build_bass(a_pad, parts=('imu', 'atom')):
    import concourse.bass as bass
    import concourse.mybir as mybir
    import concourse.tile as tile
    from concourse import bacc
    from concourse.bass import ts
    from concourse.masks import make_identity

    bf = mybir.dt.bfloat16
    f32 = mybir.dt.float32
    n_at = a_pad // 128            # atom tiles per core
    n_wt = NW // 128               # window tiles per core
    NCH = 7680 // 512              # 15 ad2 output chunks

    nc = bacc.Bacc("TRN2", target_bir_lowering=False)

    # ---- DRAM IO ----
    bridgeT = nc.dram_tensor("bridgeT", (D, NW), bf, kind="ExternalInput")
    wd1 = nc.dram_tensor("wd1", (D, 640), bf, kind="ExternalInput")
    d1b = nc.dram_tensor("d1b", (128, 5), f32, kind="ExternalInput")
    wd2 = nc.dram_tensor("wd2", (128, 5, 120), bf, kind="ExternalInput")
    d2b = nc.dram_tensor("d2b", (1, 120), f32, kind="ExternalInput")
    afT = nc.dram_tensor("afT", (128, 5, a_pad), bf, kind="ExternalInput")
    wae1 = nc.dram_tensor("wae1", (128, 5, 320), bf, kind="ExternalInput")
    ae1b = nc.dram_tensor("ae1b", (1, 320), bf, kind="ExternalInput")
    wae2a = nc.dram_tensor("wae2a", (128, 80), bf, kind="ExternalInput")
    wae2b = nc.dram_tensor("wae2b", (32, 80), bf, kind="ExternalInput")
    ae2b = nc.dram_tensor("ae2b", (80, 1), f32, kind="ExternalInput")
    wae3 = nc.dram_tensor("wae3", (80, 32), bf, kind="ExternalInput")
    ae3b = nc.dram_tensor("ae3b", (32, 1), f32, kind="ExternalInput")
    wad1 = nc.dram_tensor("wad1", (32, 256), bf, kind="ExternalInput")
    ad1b = nc.dram_tensor("ad1b", (128, 2), f32, kind="ExternalInput")
    wad2 = nc.dram_tensor("wad2", (128, 2, 7680), bf, kind="ExternalInput")
    ad2bb = nc.dram_tensor("ad2bb", (1, 7680), f32, kind="ExternalInput")
    imu_out = nc.dram_tensor("imu", (NW, 120), f32, kind="ExternalOutput")
    atom_out = nc.dram_tensor("atom", (a_pad, 7680), f32, kind="ExternalOutput")

    def bcast_ap(t, n_free):
        a = t.ap()
        return bass.AP(tensor=a.tensor, offset=a.offset,
                       ap=[[0, 128], [1, n_free]])

    with tile.TileContext(nc) as tc:
        with (
            tc.tile_pool(name="consts", bufs=1) as consts,
            tc.tile_pool(name="work", bufs=3) as work,
            tc.tile_pool(name="stage", bufs=4) as stage,
            tc.tile_pool(name="ps_small", bufs=5, space="PSUM") as ps_small,
            tc.tile_pool(name="ps_big", bufs=3, space="PSUM") as ps_big,
        ):
            # ---- constants into SBUF ----
            ident = consts.tile([128, 128], bf)
            make_identity(nc, ident)
            ones_r = consts.tile([1, 128], bf)
            nc.vector.memset(ones_r, 1.0)

            sb_bridgeT = consts.tile([D, NW], bf)
            nc.sync.dma_start(out=sb_bridgeT, in_=bridgeT[:, :])
            sb_wd1 = consts.tile([D, 640], bf)
            nc.sync.dma_start(out=sb_wd1, in_=wd1[:, :])
            sb_d1b = consts.tile([128, 5], f32)
            nc.sync.dma_start(out=sb_d1b, in_=d1b[:, :])
            sb_wd2 = consts.tile([128, 5, 120], bf)
            nc.sync.dma_start(out=sb_wd2, in_=wd2[:, :, :])
            sb_d2b = consts.tile([128, 120], f32)
            nc.sync.dma_start(out=sb_d2b, in_=bcast_ap(d2b, 120))
            sb_afT = consts.tile([128, 5, a_pad], bf)
            nc.sync.dma_start(out=sb_afT, in_=afT[:, :, :])
            sb_wae1 = consts.tile([128, 5, 320], bf)
            nc.sync.dma_start(out=sb_wae1, in_=wae1[:, :, :])
            sb_ae1b = consts.tile([1, 320], bf)
            nc.sync.dma_start(out=sb_ae1b, in_=ae1b[:, :])
            sb_wae2a = consts.tile([128, 80], bf)
            nc.sync.dma_start(out=sb_wae2a, in_=wae2a[:, :])
            sb_wae2b = consts.tile([32, 80], bf)
            nc.sync.dma_start(out=sb_wae2b, in_=wae2b[:, :])
            sb_ae2b = consts.tile([80, 1], f32)
            nc.sync.dma_start(out=sb_ae2b, in_=ae2b[:, :])
            sb_wae3 = consts.tile([80, 32], bf)
            nc.sync.dma_start(out=sb_wae3, in_=wae3[:, :])
            sb_ae3b = consts.tile([32, 1], f32)
            nc.sync.dma_start(out=sb_ae3b, in_=ae3b[:, :])
            sb_wad1 = consts.tile([32, 256], bf)
            nc.sync.dma_start(out=sb_wad1, in_=wad1[:, :])
            sb_ad1b = consts.tile([128, 2], f32)
            nc.sync.dma_start(out=sb_ad1b, in_=ad1b[:, :])
            sb_wad2 = consts.tile([128, 2, 7680], bf)
            nc.sync.dma_start(out=sb_wad2, in_=wad2[:, :, :])
            sb_ad2b = consts.tile([128, 7680], f32)
            nc.sync.dma_start(out=sb_ad2b, in_=bcast_ap(ad2bb, 7680))

            relu = mybir.ActivationFunctionType.Relu
            ident_fn = mybir.ActivationFunctionType.Identity

            # ---- imu decoder, 16 window tiles ----
            for t in range(n_wt if 'imu' in parts else 0):
                bT = sb_bridgeT[:, ts(t, 128)]
                gT = work.tile([128, 5, 128], bf, tag="gT")
                for i in range(5):
                    ps_g = ps_small.tile([128, 128], f32, tag="ps")
                    nc.tensor.matmul(ps_g, sb_wd1[:, ts(i, 128)], bT,
                                     start=True, stop=True)
                    nc.scalar.activation(gT[:, i, :], ps_g, relu,
                                         bias=sb_d1b[:, i:i + 1])
                ps_imu = ps_small.tile([128, 120], f32, tag="ps")
                for i in range(5):
                    nc.tensor.matmul(ps_imu, gT[:, i, :], sb_wd2[:, i, :],
                                     start=(i == 0), stop=(i == 4))
                imu_sb = stage.tile([128, 120], f32, tag="imu_sb")
                nc.vector.tensor_add(imu_sb, ps_imu, sb_d2b)
                nc.sync.dma_start(out=imu_out[ts(t, 128), :], in_=imu_sb)

            # ---- atom network, a_pad//128 atom tiles ----
            for t in range(n_at if 'atom' in parts else 0):
                # ae1: conv as dense window matmul, atoms on partitions
                ps1 = ps_small.tile([128, 320], f32, tag="ps")
                for i in range(5):
                    nc.tensor.matmul(ps1, sb_afT[:, i, ts(t, 128)],
                                     sb_wae1[:, i, :],
                                     start=(i == 0), stop=False)
                nc.tensor.matmul(ps1, ones_r, sb_ae1b, start=False, stop=True)
                # relu then maxpool2: [128, 320] -> [128, 160]
                s1 = work.tile([128, 320], bf, tag="s1")
                nc.scalar.activation(s1, ps1, relu)
                s3 = s1[:, :].rearrange("p (f s) -> p f s", s=2)
                h1 = work.tile([128, 160], bf, tag="h1")
                nc.vector.tensor_max(h1, s3[:, :, 0], s3[:, :, 1])
                # transpose h1 -> [160, 128] (two PE transposes)
                h1Ta = work.tile([128, 128], bf, tag="h1Ta")
                h1Tb = work.tile([32, 128], bf, tag="h1Tb")
                psta = ps_small.tile([128, 128], bf, tag="ps")
                nc.tensor.transpose(psta, h1[:, 0:128], ident)
                nc.scalar.activation(h1Ta, psta, ident_fn)
                pstb = ps_small.tile([32, 128], bf, tag="ps")
                nc.tensor.transpose(pstb, h1[:, 128:160], ident)
                nc.scalar.activation(h1Tb, pstb, ident_fn)
                # ae2: features on partitions [80, 128]
                ps2 = ps_small.tile([80, 128], f32, tag="ps")
                nc.tensor.matmul(ps2, sb_wae2a, h1Ta, start=True, stop=False)
                nc.tensor.matmul(ps2, sb_wae2b, h1Tb, start=False, stop=True)
                h2T = work.tile([80, 128], bf, tag="h2T")
                nc.scalar.activation(h2T, ps2, relu, bias=sb_ae2b[:, 0:1])
                # ae3: emb [32, 128]
                ps3 = ps_small.tile([32, 128], f32, tag="ps")
                nc.tensor.matmul(ps3, sb_wae3, h2T, start=True, stop=True)
                embT = work.tile([32, 128], bf, tag="embT")
                nc.scalar.activation(embT, ps3, ident_fn, bias=sb_ae3b[:, 0:1])
                # ad1: hdT two chunks [128, 128]
                hdT = work.tile([128, 2, 128], bf, tag="hdT")
                for j in range(2):
                    ps4 = ps_small.tile([128, 128], f32, tag="ps")
                    nc.tensor.matmul(ps4, sb_wad1[:, ts(j, 128)], embT,
                                     start=True, stop=True)
                    nc.scalar.activation(hdT[:, j, :], ps4, relu,
                                         bias=sb_ad1b[:, j:j + 1])
                # ad2: 15 chunks of 512
                for n_i in range(NCH):
                    ps5 = ps_big.tile([128, 512], f32, tag="ps5")
                    nc.tensor.matmul(ps5, hdT[:, 0, :],
                                     sb_wad2[:, 0, ts(n_i, 512)],
                                     start=True, stop=False)
                    nc.tensor.matmul(ps5, hdT[:, 1, :],
                                     sb_wad2[:, 1, ts(n_i, 512)],
                                     start=False, stop=True)
                    out_sb = stage.tile([128, 512], f32, tag="out_sb")
                    nc.vector.tensor_add(out_sb, ps5, sb_ad2b[:, ts(n_i, 512)])
                    nc.sync.dma_start(out=atom_out[ts(t, 128), ts(n_i, 512)],
                                      in_=out_sb)

    nc.compile()
    return nc


# ----------------------------------------------------------------------------
# kernel()
# ----------------------------------------------------------------------------
def kernel(x, imu_mask, c1_w, c1_b, c2_w, c2_b, c3_w, c3_b, br_w, br_b,
           f1_w, f1_b, f2_w, f2_b, d1_w, d1_b, d2_w, d2_b,
           ae1_w, ae1_b, ae2_w, ae2_b, ae3_w, ae3_b,
           ad1_w, ad1_b, ad2_w, ad2_b, imu_len):
    _ensure_ntff_hook()
    from concourse.bass_utils import run_bass_kernel_spmd

    # ---- phase 1: bit-exact replay of the reference decision path ----
    forcast_in3, forcast_in, forcast, fmask, forcast_loss = _replay_phase1(
        x, imu_mask, c1_w, c1_b, c2_w, c2_b, c3_w, c3_b,
        br_w, br_b, f1_w, f1_b, f2_w, f2_b)

    scores = np.asarray(forcast_loss * fmask[:, 0])
    k = int(scores.shape[0] * 0.2)
    cutoff = float(np.sort(scores)[::-1][:k].min())
    l2 = np.asarray(forcast_loss).reshape(BS, SEQ)
    seg_lists = _segment(l2, cutoff)

    bridge_np = np.asarray(forcast_in3)          # [bs, seq, D] f32
    x_np = np.asarray(x, dtype=np.float32)

    # flatten segments -> (b, last, e) triples in output order
    b_l, last_l, e_l = [], [], []
    for b in range(BS):
        last = 0
        for e in seg_lists[b]:
            b_l.append(b); last_l.append(last); e_l.append(e)
            last = e
    A = len(b_l)
    degenerate = A == 0
    if degenerate:
        A = 1
        b_arr = np.zeros(1, np.int64)
        last_arr = np.zeros(1, np.int64)
        e_arr = np.zeros(1, np.int64)
    else:
        b_arr = np.asarray(b_l); last_arr = np.asarray(last_l)
        e_arr = np.asarray(e_l)

    # ---- host: exact gather/pad outputs from x ----
    if degenerate:
        imu_atoms = np.zeros((1, C, WTOT), np.float32)
        imu_atoms_mask = np.zeros((1, C, WTOT), np.float32)
        af = np.zeros((1, MAL, D), np.float32)
    else:
        q = np.arange(QW)
        sidx = e_arr[:, None] - QW + q                      # [A, 64]
        valid = sidx >= last_arr[:, None]
        sc = np.clip(sidx, 0, SEQ - 1)
        xg = x_np[b_arr[:, None], sc]                       # [A, 64, 6, 20]
        xg = xg * valid[:, :, None, None].astype(np.float32)
        imu_atoms = np.ascontiguousarray(xg.transpose(0, 2, 1, 3)).reshape(A, C, WTOT)
        maskq = np.repeat(valid.astype(np.float32), L, axis=1)   # [A, 1280]
        imu_atoms_mask = np.repeat(maskq[:, None, :], C, axis=1)

        t10 = np.arange(MAL)
        fsrc = e_arr[:, None] - MAL + t10                   # [A, 10]
        fvalid = fsrc >= last_arr[:, None]
        fsc = np.clip(fsrc, 0, SEQ - 1)
        af = bridge_np[b_arr[:, None], fsc]                 # [A, 10, D]
        af = af * fvalid[:, :, None].astype(np.float32)

    # ---- build per-core bass inputs ----
    a_pad = max(128, ((A + NCORES - 1) // NCORES + 127) // 128 * 128)
    a_tot = a_pad * NCORES
    afT = np.zeros((640, a_tot), np.float32)
    # feature (d, t) = d*MAL + t
    afT[:, :A] = af.transpose(2, 1, 0).reshape(640, A)
    afT_ch = np.ascontiguousarray(afT.reshape(5, 128, a_tot).transpose(1, 0, 2))

    bridgeT = np.ascontiguousarray(
        bridge_np.reshape(N, D).T)                          # [64, 16384]

    # dense weight matrices
    # d1: g[n, o*20 + i*5 + k] = sum_c bridge[n, c*4+i] * d1_w[c, o, k]
    d1_wn = np.asarray(d1_w, np.float32)
    Wd1 = np.zeros((D, 640), np.float32)
    for c_i in range(16):
        for i in range(4):
            for o in range(32):
                for kk in range(5):
                    Wd1[c_i * 4 + i, o * 20 + i * 5 + kk] = d1_wn[c_i, o, kk]
    d1_bias_full = np.repeat(np.asarray(d1_b, np.float32), 20)      # [640]
    Wd2 = _conv_W(np.asarray(d2_w, np.float32), 20)                 # [640, 120]
    d2_bias_full = np.repeat(np.asarray(d2_b, np.float32), 20)      # [120]

    Wae1 = _conv_W(np.asarray(ae1_w, np.float32), MAL)              # [640, 320]
    ae1_bias_full = np.repeat(np.asarray(ae1_b, np.float32), MAL)   # [320]
    Wae2 = _conv_W(np.asarray(ae2_w, np.float32), 5)                # [160, 80]
    ae2_bias_full = np.repeat(np.asarray(ae2_b, np.float32), 5)     # [80]
    Wae3 = np.asarray(ae3_w, np.float32)                            # [80, 32]
    Wad1 = np.asarray(ad1_w, np.float32)                            # [32, 256]
    Wad2 = np.asarray(ad2_w, np.float32)                            # [256, 7680]

    in_common = {
        "wd1": _bf16(Wd1),
        "d1b": np.pad(d1_bias_full, (0, 0)).reshape(5, 128).T.copy(),
        "wd2": _bf16(Wd2.reshape(5, 128, 120).transpose(1, 0, 2)),
        "d2b": d2_bias_full.reshape(1, 120).copy(),
        "wae1": _bf16(Wae1.reshape(5, 128, 320).transpose(1, 0, 2)),
        "ae1b": _bf16(ae1_bias_full.reshape(1, 320)),
        "wae2a": _bf16(Wae2[:128]),
        "wae2b": _bf16(Wae2[128:]),
        "ae2b": ae2_bias_full.reshape(80, 1).copy(),
        "wae3": _bf16(Wae3),
        "ae3b": np.asarray(ae3_b, np.float32).reshape(32, 1).copy(),
        "wad1": _bf16(Wad1),
        "ad1b": np.asarray(ad1_b, np.float32).reshape(2, 128).T.copy(),
        "wad2": _bf16(Wad2.reshape(2, 128, 7680).transpose(1, 0, 2)),
        "ad2bb": np.asarray(ad2_b, np.float32).reshape(1, 7680).copy(),
    }
    in_maps = []
    for c_i in range(NCORES):
        m = dict(in_common)
        m["bridgeT"] = _bf16(bridgeT[:, c_i * NW:(c_i + 1) * NW])
        m["afT"] = _bf16(afT_ch[:, :, c_i * a_pad:(c_i + 1) * a_pad])
        in_maps.append(m)

    # ---- compile + run the bass kernel ----
    if a_pad not in _KERNEL_CACHE:
        _KERNEL_CACHE[a_pad] = _build_bass(a_pad)
    nc = _KERNEL_CACHE[a_pad]
    res = run_bass_kernel_spmd(nc, in_maps, core_ids=list(range(NCORES)))
    LAST_RESULTS.clear()
    LAST_RESULTS.append(res)

    imu_gen = np.concatenate([res.results[i]["imu"] for i in range(NCORES)],
                             axis=0).reshape(BS, SEQ, C, L)
    atom_full = np.concatenate([res.results[i]["atom"] for i in range(NCORES)],
                               axis=0)
    atom_gen = atom_full[:A].reshape(A, C, WTOT)

    bridge_resh = np.asarray(forcast_in).reshape(N, 16, 4)
    return (imu_gen, atom_gen, imu_atoms_mask, imu_atoms, bridge_resh,
            np.asarray(forcast_in), np.asarray(forcast), np.asarray(fmask),
            np.asarray(forcast_loss))


# revision 9
# speedup vs baseline: 1.0403x; 1.0403x over previous
"""Trainium2 Bass kernel for nn_AtomicHAR.

Strategy
--------
* The top-20% cutoff + per-batch segmentation in the reference is decided on
  exact f32 loss values that contain bit-level ties at the cutoff, so the
  decision path (CNN -> bridge -> forecast loss) is replayed with the exact
  same eager jax op sequence as the reference (bit-identical on this backend).
  Those bit-exact tensors also serve as the forecast-family outputs.
* The heavy generated outputs (imu decoder and the atom encoder/decoder,
  including the [A, 6, 1280] atom_gen) run in a single Bass NEFF, SPMD over
  8 NeuronCores: imu decoder data-parallel over batch, atom network
  data-parallel over atoms.
* imu_atoms / imu_atoms_mask are pure gather/pad rearrangements of the raw
  input x, assembled host-side (exact).
"""

import sys
import types

import numpy as np

MAL = 10
BS, SEQ, C, L = 128, 128, 6, 20
N = BS * SEQ
D, H = 64, 256
NCORES = 8
BS_LOC = BS // NCORES          # batches per core
NW = BS_LOC * SEQ              # windows per core
WTOT = MAL * SEQ               # 1280
QW = WTOT // L                 # 64 source windows per atom row

LAST_RESULTS = []              # BassKernelResults of the bass launch(es)


# ----------------------------------------------------------------------------
# NTFF trace hook shim: make `trace=True` (or BASS_TRACE=1) safe even when the
# image's antenv package lacks axon_hooks.
# ----------------------------------------------------------------------------
def _ensure_ntff_hook():
    try:
        import antenv.axon_hooks  # noqa: F401
        return
    except Exception:
        pass
    try:
        import antenv
    except Exception:
        return
    mod = types.ModuleType('antenv.axon_hooks')
    state = {'hook': None}
    mod.set_axon_ntff_profile_hook = lambda h: state.__setitem__('hook', h)
    mod.get_axon_ntff_profile_hook = lambda: state['hook']
    sys.modules['antenv.axon_hooks'] = mod
    antenv.axon_hooks = mod
    try:
        from trn_agent_boot.trn_boot import _ntff_profile_via_ctypes
        hook = _ntff_profile_via_ctypes('/opt/axon/libaxon_pjrt.so')
        if hook is not None:
            mod.set_axon_ntff_profile_hook(hook)
    except Exception:
        pass


# ----------------------------------------------------------------------------
# Phase 1 replay: the exact op sequence of the reference through forcast_loss.
# ----------------------------------------------------------------------------
def _replay_phase1(x, imu_mask, c1_w, c1_b, c2_w, c2_b, c3_w, c3_b,
                   br_w, br_b, f1_w, f1_b, f2_w, f2_b):
    import jax
    import jax.numpy as jnp
    from jax import lax

    def _conv1d(x_, w, b):
        pad = (w.shape[-1] - 1) // 2
        y = lax.conv_general_dilated(x_, w, (1,), [(pad, pad)],
                                     dimension_numbers=('NCH', 'OIH', 'NCH'))
        return y + b[None, :, None]

    def _maxpool2(x_):
        return lax.reduce_window(x_, -jnp.inf, lax.max, (1, 1, 2), (1, 1, 2),
                                 'VALID')

    bs, seq, Cc, Ll = x.shape
    n = bs * seq
    h = _maxpool2(jax.nn.relu(_conv1d(x.reshape(n, Cc, Ll), c1_w, c1_b)))
    h = _maxpool2(jax.nn.relu(_conv1d(h, c2_w, c2_b)))
    h = _maxpool2(jax.nn.relu(_conv1d(h, c3_w, c3_b)))
    cnn_out = h.reshape(n, -1)
    bridge_out = jax.nn.sigmoid(cnn_out @ br_w + br_b)
    d = bridge_out.shape[1]
    forcast_in3 = bridge_out.reshape(bs, seq, d)
    shft = jnp.pad(forcast_in3, ((0, 0), (1, 0), (0, 0)))[:, :-1]
    fmask = jnp.ones((bs, seq, d), jnp.float32).at[:, 0, :].set(0.0).reshape(n, d)
    fmask = fmask * imu_mask[:, :, 0, 0].reshape(n, 1)
    forcast_in = forcast_in3.reshape(n, d)
    feat = jax.nn.relu(shft.reshape(n, d) @ f1_w + f1_b)
    forcast = feat @ f2_w + f2_b
    forcast_loss = jnp.mean(jnp.square((forcast - forcast_in) * fmask), axis=1)
    return forcast_in3, forcast_in, forcast, fmask, forcast_loss


def _segment(l2, cutoff):
    """Exact mirror of the reference's host segmentation loop."""
    seg_lists = []
    for b in range(l2.shape[0]):
        idxs = np.nonzero(l2[b] > cutoff)[0]
        segs = []
        if idxs.shape[0] > 0:
            for i in range(idxs.shape[0] - 1):
                if int(idxs[i + 1] - idxs[i]) > MAL:
                    cur = int(idxs[i]); segs.append(cur)
                    while cur < int(idxs[i + 1]):
                        cur += MAL
                        if cur >= int(idxs[i + 1]):
                            break
                        segs.append(cur)
                else:
                    segs.append(int(idxs[i]))
            segs.append(int(idxs[-1]))
        seg_lists.append(segs)
    return seg_lists


# ----------------------------------------------------------------------------
# Dense per-window conv matrices (conv as matmul over the flattened window).
# ----------------------------------------------------------------------------
def _conv_W(w, L_in):
    """[I*L_in, O*L_in] dense matrix of a 'same' K-tap conv1d.
    Row (i, li) = i*L_in+li, col (o, lo) = o*L_in+lo."""
    O, I, K = w.shape
    pad = (K - 1) // 2
    W = np.zeros((I * L_in, O * L_in), np.float32)
    lo = np.arange(L_in)
    for o in range(O):
        for i in range(I):
            for k in range(K):
                li = lo + k - pad
                m = (li >= 0) & (li < L_in)
                W[i * L_in + li[m], o * L_in + lo[m]] += w[o, i, k]
    return W


def _bf16(a):
    import ml_dtypes
    return np.ascontiguousarray(np.asarray(a, np.float32)).astype(ml_dtypes.bfloat16)


# ----------------------------------------------------------------------------
# The Bass kernel: one NEFF, SPMD across 8 cores.
#   imu decoder: bridgeT [64, NW] -> imu [NW, 120]
#   atom net:    afT chunks [128, 5, A_pad] -> atom [A_pad, 7680]
# ----------------------------------------------------------------------------
_KERNEL_CACHE = {}


def _build_bass(a_pad, a_chunk, parts=('imu', 'atom')):
    import concourse.bass as bass
    import concourse.mybir as mybir
    import concourse.tile as tile
    from concourse import bacc
    from concourse.bass import ts, ds
    from concourse.masks import make_identity

    bf = mybir.dt.bfloat16
    f32 = mybir.dt.float32
    n_at = a_pad // 128            # atom tiles per core
    n_bt = NW // 512               # big window tiles per core (imu path)
    NCH = 7680 // 512              # 15 ad2 output chunks

    nc = bacc.Bacc("TRN2", target_bir_lowering=False)

    # ---- DRAM IO ----
    bridgeT = nc.dram_tensor("bridgeT", (D, NW), bf, kind="ExternalInput")
    wd1 = nc.dram_tensor("wd1", (D, 640), bf, kind="ExternalInput")
    d1b = nc.dram_tensor("d1b", (128, 5), f32, kind="ExternalInput")
    wd2 = nc.dram_tensor("wd2", (128, 5, 120), bf, kind="ExternalInput")
    afT = nc.dram_tensor("afT", (128, 5, a_pad), bf, kind="ExternalInput")
    wae1 = nc.dram_tensor("wae1", (128, 5, 320), bf, kind="ExternalInput")
    ae1b = nc.dram_tensor("ae1b", (1, 320), bf, kind="ExternalInput")
    wae2a = nc.dram_tensor("wae2a", (128, 80), bf, kind="ExternalInput")
    wae2b = nc.dram_tensor("wae2b", (32, 80), bf, kind="ExternalInput")
    ae2b = nc.dram_tensor("ae2b", (80, 1), f32, kind="ExternalInput")
    wae3 = nc.dram_tensor("wae3", (80, 32), bf, kind="ExternalInput")
    ae3b = nc.dram_tensor("ae3b", (32, 1), f32, kind="ExternalInput")
    wad1 = nc.dram_tensor("wad1", (32, 256), bf, kind="ExternalInput")
    ad1b = nc.dram_tensor("ad1b", (128, 2), f32, kind="ExternalInput")
    wad2 = nc.dram_tensor("wad2", (128, 2, 7680), bf, kind="ExternalInput")
    imu_out = nc.dram_tensor("imu", (NW, 120), bf, kind="ExternalOutput")
    atom_out = nc.dram_tensor("atom", (a_pad, 7680), bf, kind="ExternalOutput")

    relu = mybir.ActivationFunctionType.Relu
    ident_fn = mybir.ActivationFunctionType.Identity

    with tile.TileContext(nc) as tc:
        with (
            tc.tile_pool(name="consts", bufs=1) as consts,
            tc.tile_pool(name="work", bufs=3) as work,
            tc.tile_pool(name="stage", bufs=2) as stage,
            tc.tile_pool(name="psA", bufs=2, space="PSUM") as psA,
            tc.tile_pool(name="psG", bufs=2, space="PSUM") as psG,
            tc.tile_pool(name="ps5p", bufs=4, space="PSUM") as ps5p,
        ):
            # ---- small constants first: imu path can start right away ----
            sb_bridgeT = consts.tile([D, NW], bf)
            nc.sync.dma_start(out=sb_bridgeT, in_=bridgeT[:, :])
            sb_wd1 = consts.tile([D, 640], bf)
            nc.sync.dma_start(out=sb_wd1, in_=wd1[:, :])
            sb_d1b = consts.tile([128, 5], f32)
            nc.sync.dma_start(out=sb_d1b, in_=d1b[:, :])
            sb_wd2 = consts.tile([128, 5, 120], bf)
            nc.sync.dma_start(out=sb_wd2, in_=wd2[:, :, :])
            sb_afT = consts.tile([128, 5, a_pad], bf)
            nc.sync.dma_start(out=sb_afT, in_=afT[:, :, :])
            sb_wae1 = consts.tile([128, 5, 320], bf)
            nc.sync.dma_start(out=sb_wae1, in_=wae1[:, :, :])
            ident = consts.tile([128, 128], bf)
            make_identity(nc, ident)
            ones_r = consts.tile([1, 128], bf)
            nc.vector.memset(ones_r, 1.0)
            sb_ae1b = consts.tile([1, 320], bf)
            nc.sync.dma_start(out=sb_ae1b, in_=ae1b[:, :])
            sb_wae2a = consts.tile([128, 80], bf)
            nc.sync.dma_start(out=sb_wae2a, in_=wae2a[:, :])
            sb_wae2b = consts.tile([32, 80], bf)
            nc.sync.dma_start(out=sb_wae2b, in_=wae2b[:, :])
            sb_ae2b = consts.tile([80, 1], f32)
            nc.sync.dma_start(out=sb_ae2b, in_=ae2b[:, :])
            sb_wae3 = consts.tile([80, 32], bf)
            nc.sync.dma_start(out=sb_wae3, in_=wae3[:, :])
            sb_ae3b = consts.tile([32, 1], f32)
            nc.sync.dma_start(out=sb_ae3b, in_=ae3b[:, :])
            sb_wad1 = consts.tile([32, 256], bf)
            nc.sync.dma_start(out=sb_wad1, in_=wad1[:, :])
            sb_ad1b = consts.tile([128, 2], f32)
            nc.sync.dma_start(out=sb_ad1b, in_=ad1b[:, :])
            # the big ad2 weight goes last: only needed once ad2 begins
            sb_wad2 = consts.tile([128, 2, 7680], bf)
            nc.sync.dma_start(out=sb_wad2, in_=wad2[:, :, :])

            # ---- imu decoder, 4 big window tiles of 512 ----
            for t in range(n_bt if 'imu' in parts else 0):
                bT = sb_bridgeT[:, ts(t, 512)]
                gT = work.tile([128, 5, 512], bf, tag="gT")
                for i in range(5):
                    ps_g = psG.tile([128, 512], f32, tag="ps_g")
                    nc.tensor.matmul(ps_g, sb_wd1[:, ts(i, 128)], bT,
                                     start=True, stop=True)
                    nc.scalar.activation(gT[:, i, :], ps_g, relu,
                                         bias=sb_d1b[:, i:i + 1])
                imu_sb = stage.tile([128, 4, 120], bf, tag="imu_sb")
                for sub in range(4):
                    ps_imu = psA.tile([128, 120], f32, tag="ps")
                    for i in range(5):
                        nc.tensor.matmul(ps_imu,
                                         gT[:, i, ds(sub * 128, 128)],
                                         sb_wd2[:, i, :],
                                         start=(i == 0), stop=(i == 4))
                    nc.vector.tensor_copy(imu_sb[:, sub, :], ps_imu)
                # one DMA per big tile: [128, 4, 120] -> rows t*512 .. +512
                dst = imu_out.ap()
                dst = bass.AP(
                    tensor=dst.tensor, offset=dst.offset + t * 512 * 120,
                    ap=[[120, 128], [128 * 120, 4], [1, 120]])
                nc.sync.dma_start(out=dst, in_=imu_sb)

            # ---- atom network, a_pad//128 atom tiles ----
            evict_i = 0
            for t in range(n_at if 'atom' in parts else 0):
                # ae1: conv as dense window matmul, atoms on partitions
                ps1 = psA.tile([128, 320], f32, tag="ps")
                for i in range(5):
                    nc.tensor.matmul(ps1, sb_afT[:, i, ts(t, 128)],
                                     sb_wae1[:, i, :],
                                     start=(i == 0), stop=False)
                nc.tensor.matmul(ps1, ones_r, sb_ae1b, start=False, stop=True)
                # relu then maxpool2: [128, 320] -> [128, 160]
                s1 = work.tile([128, 320], bf, tag="s1")
                nc.scalar.activation(s1, ps1, relu)
                s3 = s1[:, :].rearrange("p (f s) -> p f s", s=2)
                h1 = work.tile([128, 160], bf, tag="h1")
                nc.vector.tensor_max(h1, s3[:, :, 0], s3[:, :, 1])
                # transpose h1 -> [160, 128] (two PE transposes)
                h1Ta = work.tile([128, 128], bf, tag="h1Ta")
                h1Tb = work.tile([32, 128], bf, tag="h1Tb")
                psta = psA.tile([128, 128], bf, tag="ps")
                nc.tensor.transpose(psta, h1[:, 0:128], ident)
                nc.scalar.copy(h1Ta, psta)
                pstb = psA.tile([32, 128], bf, tag="ps")
                nc.tensor.transpose(pstb, h1[:, 128:160], ident)
                nc.scalar.copy(h1Tb, pstb)
                # ae2: features on partitions [80, 128]
                ps2 = psA.tile([80, 128], f32, tag="ps")
                nc.tensor.matmul(ps2, sb_wae2a, h1Ta, start=True, stop=False)
                nc.tensor.matmul(ps2, sb_wae2b, h1Tb, start=False, stop=True)
                h2T = work.tile([80, 128], bf, tag="h2T")
                nc.scalar.activation(h2T, ps2, relu, bias=sb_ae2b[:, 0:1])
                # ae3: emb [32, 128]
                ps3 = psA.tile([32, 128], f32, tag="ps")
                nc.tensor.matmul(ps3, sb_wae3, h2T, start=True, stop=True)
                embT = work.tile([32, 128], bf, tag="embT")
                nc.scalar.activation(embT, ps3, ident_fn, bias=sb_ae3b[:, 0:1])
                # ad1: hdT two chunks [128, 128]
                hdT = work.tile([128, 2, 128], bf, tag="hdT")
                for j in range(2):
                    ps4 = psA.tile([128, 128], f32, tag="ps")
                    nc.tensor.matmul(ps4, sb_wad1[:, ts(j, 128)], embT,
                                     start=True, stop=True)
                    nc.scalar.activation(hdT[:, j, :], ps4, relu,
                                         bias=sb_ad1b[:, j:j + 1])
                # ad2: 15 chunks of 512 (bias folded on host); full-row staging
                rows = min(128, max(0, a_chunk - t * 128))
                out_sb = stage.tile([128, 7680], bf, tag="out_sb")
                for n_i in range(NCH):
                    ps5 = ps5p.tile([128, 512], f32, tag="ps5")
                    nc.tensor.matmul(ps5, hdT[:, 0, :],
                                     sb_wad2[:, 0, ts(n_i, 512)],
                                     start=True, stop=False)
                    nc.tensor.matmul(ps5, hdT[:, 1, :],
                                     sb_wad2[:, 1, ts(n_i, 512)],
                                     start=False, stop=True)
                    if evict_i % 2 == 0:
                        nc.vector.tensor_copy(out_sb[:, ts(n_i, 512)], ps5)
                    else:
                        nc.scalar.copy(out_sb[:, ts(n_i, 512)], ps5)
                    evict_i += 1
                if rows > 0:
                    nc.sync.dma_start(
                        out=atom_out[ds(t * 128, rows), :],
                        in_=out_sb[0:rows, :])

    nc.compile()
    return nc


# ----------------------------------------------------------------------------
# kernel()
# ----------------------------------------------------------------------------
def kernel(x, imu_mask, c1_w, c1_b, c2_w, c2_b, c3_w, c3_b, br_w, br_b,
           f1_w, f1_b, f2_w, f2_b, d1_w, d1_b, d2_w, d2_b,
           ae1_w, ae1_b, ae2_w, ae2_b, ae3_w, ae3_b,
           ad1_w, ad1_b, ad2_w, ad2_b, imu_len):
    _ensure_ntff_hook()
    from concourse.bass_utils import run_bass_kernel_spmd

    # ---- phase 1: bit-exact replay of the reference decision path ----
    forcast_in3, forcast_in, forcast, fmask, forcast_loss = _replay_phase1(
        x, imu_mask, c1_w, c1_b, c2_w, c2_b, c3_w, c3_b,
        br_w, br_b, f1_w, f1_b, f2_w, f2_b)

    scores = np.asarray(forcast_loss * fmask[:, 0])
    k = int(scores.shape[0] * 0.2)
    cutoff = float(np.sort(scores)[::-1][:k].min())
    l2 = np.asarray(forcast_loss).reshape(BS, SEQ)
    seg_lists = _segment(l2, cutoff)

    bridge_np = np.asarray(forcast_in3)          # [bs, seq, D] f32
    x_np = np.asarray(x, dtype=np.float32)

    # flatten segments -> (b, last, e) triples in output order
    b_l, last_l, e_l = [], [], []
    for b in range(BS):
        last = 0
        for e in seg_lists[b]:
            b_l.append(b); last_l.append(last); e_l.append(e)
            last = e
    A = len(b_l)
    degenerate = A == 0
    if degenerate:
        A = 1
        b_arr = np.zeros(1, np.int64)
        last_arr = np.zeros(1, np.int64)
        e_arr = np.zeros(1, np.int64)
    else:
        b_arr = np.asarray(b_l); last_arr = np.asarray(last_l)
        e_arr = np.asarray(e_l)

    # ---- host: exact gather/pad outputs from x ----
    if degenerate:
        imu_atoms = np.zeros((1, C, WTOT), np.float32)
        imu_atoms_mask = np.zeros((1, C, WTOT), np.float32)
        af = np.zeros((1, MAL, D), np.float32)
    else:
        q = np.arange(QW)
        sidx = e_arr[:, None] - QW + q                      # [A, 64]
        valid = sidx >= last_arr[:, None]
        sc = np.clip(sidx, 0, SEQ - 1)
        xg = x_np[b_arr[:, None], sc]                       # [A, 64, 6, 20]
        xg = xg * valid[:, :, None, None].astype(np.float32)
        imu_atoms = np.ascontiguousarray(xg.transpose(0, 2, 1, 3)).reshape(A, C, WTOT)
        maskq = np.repeat(valid.astype(np.float32), L, axis=1)   # [A, 1280]
        imu_atoms_mask = np.repeat(maskq[:, None, :], C, axis=1)

        t10 = np.arange(MAL)
        fsrc = e_arr[:, None] - MAL + t10                   # [A, 10]
        fvalid = fsrc >= last_arr[:, None]
        fsc = np.clip(fsrc, 0, SEQ - 1)
        af = bridge_np[b_arr[:, None], fsc]                 # [A, 10, D]
        af = af * fvalid[:, :, None].astype(np.float32)

    # ---- build per-core bass inputs ----
    a_chunk = (A + NCORES - 1) // NCORES
    a_pad = max(128, (a_chunk + 127) // 128 * 128)
    a_tot = a_pad * NCORES
    # core c owns atoms [c*a_chunk, (c+1)*a_chunk), zero-padded to a_pad
    afT = np.zeros((640, a_tot), np.float32)
    afA = af.transpose(2, 1, 0).reshape(640, A)   # feature (d, t) = d*MAL + t
    for c_i in range(NCORES):
        lo = c_i * a_chunk
        hi = min(lo + a_chunk, A)
        if hi > lo:
            afT[:, c_i * a_pad:c_i * a_pad + (hi - lo)] = afA[:, lo:hi]
    afT_ch = np.ascontiguousarray(afT.reshape(5, 128, a_tot).transpose(1, 0, 2))

    bridgeT = np.ascontiguousarray(
        bridge_np.reshape(N, D).T)                          # [64, 16384]

    # dense weight matrices
    # d1: g[n, o*20 + i*5 + k] = sum_c bridge[n, c*4+i] * d1_w[c, o, k]
    d1_wn = np.asarray(d1_w, np.float32)
    Wd1 = np.zeros((D, 640), np.float32)
    for c_i in range(16):
        for i in range(4):
            for o in range(32):
                for kk in range(5):
                    Wd1[c_i * 4 + i, o * 20 + i * 5 + kk] = d1_wn[c_i, o, kk]
    d1_bias_full = np.repeat(np.asarray(d1_b, np.float32), 20)      # [640]
    Wd2 = _conv_W(np.asarray(d2_w, np.float32), 20)                 # [640, 120]
    d2_bias_full = np.repeat(np.asarray(d2_b, np.float32), 20)      # [120]

    Wae1 = _conv_W(np.asarray(ae1_w, np.float32), MAL)              # [640, 320]
    ae1_bias_full = np.repeat(np.asarray(ae1_b, np.float32), MAL)   # [320]
    Wae2 = _conv_W(np.asarray(ae2_w, np.float32), 5)                # [160, 80]
    ae2_bias_full = np.repeat(np.asarray(ae2_b, np.float32), 5)     # [80]
    Wae3 = np.asarray(ae3_w, np.float32)                            # [80, 32]
    Wad1 = np.asarray(ad1_w, np.float32)                            # [32, 256]
    Wad2 = np.asarray(ad2_w, np.float32)                            # [256, 7680]

    in_common = {
        "wd1": _bf16(Wd1),
        "d1b": d1_bias_full.reshape(5, 128).T.copy(),
        "wd2": _bf16(Wd2.reshape(5, 128, 120).transpose(1, 0, 2)),
        "wae1": _bf16(Wae1.reshape(5, 128, 320).transpose(1, 0, 2)),
        "ae1b": _bf16(ae1_bias_full.reshape(1, 320)),
        "wae2a": _bf16(Wae2[:128]),
        "wae2b": _bf16(Wae2[128:]),
        "ae2b": ae2_bias_full.reshape(80, 1).copy(),
        "wae3": _bf16(Wae3),
        "ae3b": np.asarray(ae3_b, np.float32).reshape(32, 1).copy(),
        "wad1": _bf16(Wad1),
        "ad1b": np.asarray(ad1_b, np.float32).reshape(2, 128).T.copy(),
        "wad2": _bf16(Wad2.reshape(2, 128, 7680).transpose(1, 0, 2)),
    }
    in_maps = []
    for c_i in range(NCORES):
        m = dict(in_common)
        m["bridgeT"] = _bf16(bridgeT[:, c_i * NW:(c_i + 1) * NW])
        m["afT"] = _bf16(afT_ch[:, :, c_i * a_pad:(c_i + 1) * a_pad])
        in_maps.append(m)

    # ---- compile + run the bass kernel ----
    key = (a_pad, a_chunk)
    if key not in _KERNEL_CACHE:
        _KERNEL_CACHE[key] = _build_bass(a_pad, a_chunk)
    nc = _KERNEL_CACHE[key]
    res = run_bass_kernel_spmd(nc, in_maps, core_ids=list(range(NCORES)))
    LAST_RESULTS.clear()
    LAST_RESULTS.append(res)

    imu_gen = np.concatenate(
        [res.results[i]["imu"].astype(np.float32) for i in range(NCORES)],
        axis=0)
    imu_gen = (imu_gen + d2_bias_full).reshape(BS, SEQ, C, L)
    atom_full = np.concatenate(
        [res.results[i]["atom"][:a_chunk].astype(np.float32)
         for i in range(NCORES)], axis=0)
    ad2_bn = np.asarray(ad2_b, np.float32).reshape(7680)
    atom_gen = (atom_full[:A] + ad2_bn).reshape(A, C, WTOT)

    bridge_resh = np.asarray(forcast_in).reshape(N, 16, 4)
    return (imu_gen, atom_gen, imu_atoms_mask, imu_atoms, bridge_resh,
            np.asarray(forcast_in), np.asarray(forcast), np.asarray(fmask),
            np.asarray(forcast_loss))


# revision 10
# speedup vs baseline: 1.4060x; 1.3515x over previous
"""Trainium2 Bass kernel for nn_AtomicHAR.

Strategy
--------
* The top-20% cutoff + per-batch segmentation in the reference is decided on
  exact f32 loss values that contain bit-level ties at the cutoff, so the
  decision path (CNN -> bridge -> forecast loss) is replayed with the exact
  same eager jax op sequence as the reference (bit-identical on this backend).
  Those bit-exact tensors also serve as the forecast-family outputs.
* The heavy generated outputs (imu decoder and the atom encoder/decoder,
  including the [A, 6, 1280] atom_gen) run in a single Bass NEFF, SPMD over
  8 NeuronCores: imu decoder data-parallel over batch, atom network
  data-parallel over atoms.
* imu_atoms / imu_atoms_mask are pure gather/pad rearrangements of the raw
  input x, assembled host-side (exact).
"""

import sys
import types

import numpy as np

MAL = 10
BS, SEQ, C, L = 128, 128, 6, 20
N = BS * SEQ
D, H = 64, 256
NCORES = 8
BS_LOC = BS // NCORES          # batches per core
NW = BS_LOC * SEQ              # windows per core
WTOT = MAL * SEQ               # 1280
QW = WTOT // L                 # 64 source windows per atom row

LAST_RESULTS = []              # BassKernelResults of the bass launch(es)


# ----------------------------------------------------------------------------
# NTFF trace hook shim: make `trace=True` (or BASS_TRACE=1) safe even when the
# image's antenv package lacks axon_hooks.
# ----------------------------------------------------------------------------
def _ensure_ntff_hook():
    try:
        import antenv.axon_hooks  # noqa: F401
        return
    except Exception:
        pass
    try:
        import antenv
    except Exception:
        return
    mod = types.ModuleType('antenv.axon_hooks')
    state = {'hook': None}
    mod.set_axon_ntff_profile_hook = lambda h: state.__setitem__('hook', h)
    mod.get_axon_ntff_profile_hook = lambda: state['hook']
    sys.modules['antenv.axon_hooks'] = mod
    antenv.axon_hooks = mod
    try:
        from trn_agent_boot.trn_boot import _ntff_profile_via_ctypes
        hook = _ntff_profile_via_ctypes('/opt/axon/libaxon_pjrt.so')
        if hook is not None:
            mod.set_axon_ntff_profile_hook(hook)
    except Exception:
        pass


# ----------------------------------------------------------------------------
# Phase 1 replay: the exact op sequence of the reference through forcast_loss.
# ----------------------------------------------------------------------------
def _replay_phase1(x, imu_mask, c1_w, c1_b, c2_w, c2_b, c3_w, c3_b,
                   br_w, br_b, f1_w, f1_b, f2_w, f2_b):
    import jax
    import jax.numpy as jnp
    from jax import lax

    def _conv1d(x_, w, b):
        pad = (w.shape[-1] - 1) // 2
        y = lax.conv_general_dilated(x_, w, (1,), [(pad, pad)],
                                     dimension_numbers=('NCH', 'OIH', 'NCH'))
        return y + b[None, :, None]

    def _maxpool2(x_):
        return lax.reduce_window(x_, -jnp.inf, lax.max, (1, 1, 2), (1, 1, 2),
                                 'VALID')

    bs, seq, Cc, Ll = x.shape
    n = bs * seq
    h = _maxpool2(jax.nn.relu(_conv1d(x.reshape(n, Cc, Ll), c1_w, c1_b)))
    h = _maxpool2(jax.nn.relu(_conv1d(h, c2_w, c2_b)))
    h = _maxpool2(jax.nn.relu(_conv1d(h, c3_w, c3_b)))
    cnn_out = h.reshape(n, -1)
    bridge_out = jax.nn.sigmoid(cnn_out @ br_w + br_b)
    d = bridge_out.shape[1]
    forcast_in3 = bridge_out.reshape(bs, seq, d)
    shft = jnp.pad(forcast_in3, ((0, 0), (1, 0), (0, 0)))[:, :-1]
    fmask = jnp.ones((bs, seq, d), jnp.float32).at[:, 0, :].set(0.0).reshape(n, d)
    fmask = fmask * imu_mask[:, :, 0, 0].reshape(n, 1)
    forcast_in = forcast_in3.reshape(n, d)
    feat = jax.nn.relu(shft.reshape(n, d) @ f1_w + f1_b)
    forcast = feat @ f2_w + f2_b
    forcast_loss = jnp.mean(jnp.square((forcast - forcast_in) * fmask), axis=1)
    return forcast_in3, forcast_in, forcast, fmask, forcast_loss


def _segment(l2, cutoff):
    """Exact mirror of the reference's host segmentation loop."""
    seg_lists = []
    for b in range(l2.shape[0]):
        idxs = np.nonzero(l2[b] > cutoff)[0]
        segs = []
        if idxs.shape[0] > 0:
            for i in range(idxs.shape[0] - 1):
                if int(idxs[i + 1] - idxs[i]) > MAL:
                    cur = int(idxs[i]); segs.append(cur)
                    while cur < int(idxs[i + 1]):
                        cur += MAL
                        if cur >= int(idxs[i + 1]):
                            break
                        segs.append(cur)
                else:
                    segs.append(int(idxs[i]))
            segs.append(int(idxs[-1]))
        seg_lists.append(segs)
    return seg_lists


# ----------------------------------------------------------------------------
# Dense per-window conv matrices (conv as matmul over the flattened window).
# ----------------------------------------------------------------------------
def _conv_W(w, L_in):
    """[I*L_in, O*L_in] dense matrix of a 'same' K-tap conv1d.
    Row (i, li) = i*L_in+li, col (o, lo) = o*L_in+lo."""
    O, I, K = w.shape
    pad = (K - 1) // 2
    W = np.zeros((I * L_in, O * L_in), np.float32)
    lo = np.arange(L_in)
    for o in range(O):
        for i in range(I):
            for k in range(K):
                li = lo + k - pad
                m = (li >= 0) & (li < L_in)
                W[i * L_in + li[m], o * L_in + lo[m]] += w[o, i, k]
    return W


def _bf16(a):
    import ml_dtypes
    return np.ascontiguousarray(np.asarray(a, np.float32)).astype(ml_dtypes.bfloat16)


# ----------------------------------------------------------------------------
# The Bass kernel: one NEFF, SPMD across 8 cores.
#   imu decoder: bridgeT [64, NW] -> imu [NW, 120]
#   atom net:    afT chunks [128, 5, A_pad] -> atom [A_pad, 7680]
# ----------------------------------------------------------------------------
_KERNEL_CACHE = {}


def _build_bass(a_pad, a_chunk, parts=('imu', 'atom')):
    import concourse.bass as bass
    import concourse.mybir as mybir
    import concourse.tile as tile
    from concourse import bacc
    from concourse.bass import ts, ds
    from concourse.masks import make_identity

    bf = mybir.dt.bfloat16
    f32 = mybir.dt.float32
    n_at = a_pad // 128            # atom tiles per core
    n_bt = NW // 512               # big window tiles per core (imu path)
    NCH = 7680 // 512              # 15 ad2 output chunks

    nc = bacc.Bacc("TRN2", target_bir_lowering=False)

    # ---- DRAM IO ----
    bridgeT = nc.dram_tensor("bridgeT", (D, NW), bf, kind="ExternalInput")
    wd1 = nc.dram_tensor("wd1", (D, 640), bf, kind="ExternalInput")
    d1b = nc.dram_tensor("d1b", (128, 5), f32, kind="ExternalInput")
    wd2 = nc.dram_tensor("wd2", (128, 5, 120), bf, kind="ExternalInput")
    afT = nc.dram_tensor("afT", (128, 5, a_pad), bf, kind="ExternalInput")
    wae1 = nc.dram_tensor("wae1", (128, 5, 320), bf, kind="ExternalInput")
    ae1b = nc.dram_tensor("ae1b", (1, 320), bf, kind="ExternalInput")
    wae2a = nc.dram_tensor("wae2a", (128, 80), bf, kind="ExternalInput")
    wae2b = nc.dram_tensor("wae2b", (32, 80), bf, kind="ExternalInput")
    ae2b = nc.dram_tensor("ae2b", (80, 1), f32, kind="ExternalInput")
    wae3 = nc.dram_tensor("wae3", (80, 32), bf, kind="ExternalInput")
    ae3b = nc.dram_tensor("ae3b", (32, 1), f32, kind="ExternalInput")
    wad1 = nc.dram_tensor("wad1", (32, 256), bf, kind="ExternalInput")
    ad1b = nc.dram_tensor("ad1b", (128, 2), f32, kind="ExternalInput")
    wad2 = nc.dram_tensor("wad2", (128, 2, 7680), bf, kind="ExternalInput")
    imu_out = nc.dram_tensor("imu", (NW, 120), bf, kind="ExternalOutput")
    atom_out = nc.dram_tensor("atom", (a_pad, 7680), bf, kind="ExternalOutput")

    relu = mybir.ActivationFunctionType.Relu
    ident_fn = mybir.ActivationFunctionType.Identity

    with tile.TileContext(nc) as tc:
        with (
            tc.tile_pool(name="consts", bufs=1) as consts,
            tc.tile_pool(name="work", bufs=3) as work,
            tc.tile_pool(name="stage", bufs=2) as stage,
            tc.tile_pool(name="psA", bufs=2, space="PSUM") as psA,
            tc.tile_pool(name="psG", bufs=2, space="PSUM") as psG,
            tc.tile_pool(name="ps5p", bufs=4, space="PSUM") as ps5p,
        ):
            # ---- small constants first: imu path can start right away ----
            sb_bridgeT = consts.tile([D, NW], bf)
            nc.sync.dma_start(out=sb_bridgeT, in_=bridgeT[:, :])
            sb_wd1 = consts.tile([D, 640], bf)
            nc.sync.dma_start(out=sb_wd1, in_=wd1[:, :])
            sb_d1b = consts.tile([128, 5], f32)
            nc.sync.dma_start(out=sb_d1b, in_=d1b[:, :])
            sb_wd2 = consts.tile([128, 5, 120], bf)
            nc.sync.dma_start(out=sb_wd2, in_=wd2[:, :, :])
            sb_afT = consts.tile([128, 5, a_pad], bf)
            nc.sync.dma_start(out=sb_afT, in_=afT[:, :, :])
            sb_wae1 = consts.tile([128, 5, 320], bf)
            nc.sync.dma_start(out=sb_wae1, in_=wae1[:, :, :])
            ident = consts.tile([128, 128], bf)
            make_identity(nc, ident)
            ones_r = consts.tile([1, 128], bf)
            nc.vector.memset(ones_r, 1.0)
            sb_ae1b = consts.tile([1, 320], bf)
            nc.sync.dma_start(out=sb_ae1b, in_=ae1b[:, :])
            sb_wae2a = consts.tile([128, 80], bf)
            nc.sync.dma_start(out=sb_wae2a, in_=wae2a[:, :])
            sb_wae2b = consts.tile([32, 80], bf)
            nc.sync.dma_start(out=sb_wae2b, in_=wae2b[:, :])
            sb_ae2b = consts.tile([80, 1], f32)
            nc.sync.dma_start(out=sb_ae2b, in_=ae2b[:, :])
            sb_wae3 = consts.tile([80, 32], bf)
            nc.sync.dma_start(out=sb_wae3, in_=wae3[:, :])
            sb_ae3b = consts.tile([32, 1], f32)
            nc.sync.dma_start(out=sb_ae3b, in_=ae3b[:, :])
            sb_wad1 = consts.tile([32, 256], bf)
            nc.sync.dma_start(out=sb_wad1, in_=wad1[:, :])
            sb_ad1b = consts.tile([128, 2], f32)
            nc.sync.dma_start(out=sb_ad1b, in_=ad1b[:, :])
            # the big ad2 weight goes last: only needed once ad2 begins
            sb_wad2 = consts.tile([128, 2, 7680], bf)
            nc.sync.dma_start(out=sb_wad2, in_=wad2[:, :, :])

            # ---- imu decoder, 4 big window tiles of 512 ----
            for t in range(n_bt if 'imu' in parts else 0):
                bT = sb_bridgeT[:, ts(t, 512)]
                gT = work.tile([128, 5, 512], bf, tag="gT")
                for i in range(5):
                    ps_g = psG.tile([128, 512], f32, tag="ps_g")
                    nc.tensor.matmul(ps_g, sb_wd1[:, ts(i, 128)], bT,
                                     start=True, stop=True)
                    nc.scalar.activation(gT[:, i, :], ps_g, relu,
                                         bias=sb_d1b[:, i:i + 1])
                imu_sb = stage.tile([128, 4, 120], bf, tag="imu_sb")
                for sub in range(4):
                    ps_imu = psA.tile([128, 120], f32, tag="ps")
                    for i in range(5):
                        nc.tensor.matmul(ps_imu,
                                         gT[:, i, ds(sub * 128, 128)],
                                         sb_wd2[:, i, :],
                                         start=(i == 0), stop=(i == 4))
                    nc.vector.tensor_copy(imu_sb[:, sub, :], ps_imu)
                # one DMA per big tile: [128, 4, 120] -> rows t*512 .. +512
                dst = imu_out.ap()
                dst = bass.AP(
                    tensor=dst.tensor, offset=dst.offset + t * 512 * 120,
                    ap=[[120, 128], [128 * 120, 4], [1, 120]])
                nc.sync.dma_start(out=dst, in_=imu_sb)

            # ---- atom network, a_pad//128 atom tiles ----
            evict_i = 0
            for t in range(n_at if 'atom' in parts else 0):
                # ae1: conv as dense window matmul, atoms on partitions
                ps1 = psA.tile([128, 320], f32, tag="ps")
                for i in range(5):
                    nc.tensor.matmul(ps1, sb_afT[:, i, ts(t, 128)],
                                     sb_wae1[:, i, :],
                                     start=(i == 0), stop=False)
                nc.tensor.matmul(ps1, ones_r, sb_ae1b, start=False, stop=True)
                # relu then maxpool2: [128, 320] -> [128, 160]
                s1 = work.tile([128, 320], bf, tag="s1")
                nc.scalar.activation(s1, ps1, relu)
                s3 = s1[:, :].rearrange("p (f s) -> p f s", s=2)
                h1 = work.tile([128, 160], bf, tag="h1")
                nc.vector.tensor_max(h1, s3[:, :, 0], s3[:, :, 1])
                # transpose h1 -> [160, 128] (two PE transposes)
                h1Ta = work.tile([128, 128], bf, tag="h1Ta")
                h1Tb = work.tile([32, 128], bf, tag="h1Tb")
                psta = psA.tile([128, 128], bf, tag="ps")
                nc.tensor.transpose(psta, h1[:, 0:128], ident)
                nc.scalar.copy(h1Ta, psta)
                pstb = psA.tile([32, 128], bf, tag="ps")
                nc.tensor.transpose(pstb, h1[:, 128:160], ident)
                nc.scalar.copy(h1Tb, pstb)
                # ae2: features on partitions [80, 128]
                ps2 = psA.tile([80, 128], f32, tag="ps")
                nc.tensor.matmul(ps2, sb_wae2a, h1Ta, start=True, stop=False)
                nc.tensor.matmul(ps2, sb_wae2b, h1Tb, start=False, stop=True)
                h2T = work.tile([80, 128], bf, tag="h2T")
                nc.scalar.activation(h2T, ps2, relu, bias=sb_ae2b[:, 0:1])
                # ae3: emb [32, 128]
                ps3 = psA.tile([32, 128], f32, tag="ps")
                nc.tensor.matmul(ps3, sb_wae3, h2T, start=True, stop=True)
                embT = work.tile([32, 128], bf, tag="embT")
                nc.scalar.activation(embT, ps3, ident_fn, bias=sb_ae3b[:, 0:1])
                # ad1: hdT two chunks [128, 128]
                hdT = work.tile([128, 2, 128], bf, tag="hdT")
                for j in range(2):
                    ps4 = psA.tile([128, 128], f32, tag="ps")
                    nc.tensor.matmul(ps4, sb_wad1[:, ts(j, 128)], embT,
                                     start=True, stop=True)
                    nc.scalar.activation(hdT[:, j, :], ps4, relu,
                                         bias=sb_ad1b[:, j:j + 1])
                # ad2: 15 chunks of 512 (bias folded on host); full-row staging
                out_sb = stage.tile([128, 7680], bf, tag="out_sb")
                for n_i in range(NCH):
                    ps5 = ps5p.tile([128, 512], f32, tag="ps5")
                    nc.tensor.matmul(ps5, hdT[:, 0, :],
                                     sb_wad2[:, 0, ts(n_i, 512)],
                                     start=True, stop=False)
                    nc.tensor.matmul(ps5, hdT[:, 1, :],
                                     sb_wad2[:, 1, ts(n_i, 512)],
                                     start=False, stop=True)
                    if evict_i % 2 == 0:
                        nc.vector.tensor_copy(out_sb[:, ts(n_i, 512)], ps5)
                    else:
                        nc.scalar.copy(out_sb[:, ts(n_i, 512)], ps5)
                    evict_i += 1
                # always 128 rows: partial-partition DMAs collapse onto a
                # single SDMA engine (~27 GB/s); junk rows are discarded on host
                nc.sync.dma_start(out=atom_out[ts(t, 128), :], in_=out_sb)

    nc.compile()
    return nc


# ----------------------------------------------------------------------------
# kernel()
# ----------------------------------------------------------------------------
def kernel(x, imu_mask, c1_w, c1_b, c2_w, c2_b, c3_w, c3_b, br_w, br_b,
           f1_w, f1_b, f2_w, f2_b, d1_w, d1_b, d2_w, d2_b,
           ae1_w, ae1_b, ae2_w, ae2_b, ae3_w, ae3_b,
           ad1_w, ad1_b, ad2_w, ad2_b, imu_len):
    _ensure_ntff_hook()
    from concourse.bass_utils import run_bass_kernel_spmd

    # ---- phase 1: bit-exact replay of the reference decision path ----
    forcast_in3, forcast_in, forcast, fmask, forcast_loss = _replay_phase1(
        x, imu_mask, c1_w, c1_b, c2_w, c2_b, c3_w, c3_b,
        br_w, br_b, f1_w, f1_b, f2_w, f2_b)

    scores = np.asarray(forcast_loss * fmask[:, 0])
    k = int(scores.shape[0] * 0.2)
    cutoff = float(np.sort(scores)[::-1][:k].min())
    l2 = np.asarray(forcast_loss).reshape(BS, SEQ)
    seg_lists = _segment(l2, cutoff)

    bridge_np = np.asarray(forcast_in3)          # [bs, seq, D] f32
    x_np = np.asarray(x, dtype=np.float32)

    # flatten segments -> (b, last, e) triples in output order
    b_l, last_l, e_l = [], [], []
    for b in range(BS):
        last = 0
        for e in seg_lists[b]:
            b_l.append(b); last_l.append(last); e_l.append(e)
            last = e
    A = len(b_l)
    degenerate = A == 0
    if degenerate:
        A = 1
        b_arr = np.zeros(1, np.int64)
        last_arr = np.zeros(1, np.int64)
        e_arr = np.zeros(1, np.int64)
    else:
        b_arr = np.asarray(b_l); last_arr = np.asarray(last_l)
        e_arr = np.asarray(e_l)

    # ---- host: exact gather/pad outputs from x ----
    if degenerate:
        imu_atoms = np.zeros((1, C, WTOT), np.float32)
        imu_atoms_mask = np.zeros((1, C, WTOT), np.float32)
        af = np.zeros((1, MAL, D), np.float32)
    else:
        q = np.arange(QW)
        sidx = e_arr[:, None] - QW + q                      # [A, 64]
        valid = sidx >= last_arr[:, None]
        sc = np.clip(sidx, 0, SEQ - 1)
        xg = x_np[b_arr[:, None], sc]                       # [A, 64, 6, 20]
        xg = xg * valid[:, :, None, None].astype(np.float32)
        imu_atoms = np.ascontiguousarray(xg.transpose(0, 2, 1, 3)).reshape(A, C, WTOT)
        maskq = np.repeat(valid.astype(np.float32), L, axis=1)   # [A, 1280]
        imu_atoms_mask = np.repeat(maskq[:, None, :], C, axis=1)

        t10 = np.arange(MAL)
        fsrc = e_arr[:, None] - MAL + t10                   # [A, 10]
        fvalid = fsrc >= last_arr[:, None]
        fsc = np.clip(fsrc, 0, SEQ - 1)
        af = bridge_np[b_arr[:, None], fsc]                 # [A, 10, D]
        af = af * fvalid[:, :, None].astype(np.float32)

    # ---- build per-core bass inputs ----
    a_chunk = (A + NCORES - 1) // NCORES
    a_pad = max(128, (a_chunk + 127) // 128 * 128)
    a_tot = a_pad * NCORES
    # core c owns atoms [c*a_chunk, (c+1)*a_chunk), zero-padded to a_pad
    afT = np.zeros((640, a_tot), np.float32)
    afA = af.transpose(2, 1, 0).reshape(640, A)   # feature (d, t) = d*MAL + t
    for c_i in range(NCORES):
        lo = c_i * a_chunk
        hi = min(lo + a_chunk, A)
        if hi > lo:
            afT[:, c_i * a_pad:c_i * a_pad + (hi - lo)] = afA[:, lo:hi]
    afT_ch = np.ascontiguousarray(afT.reshape(5, 128, a_tot).transpose(1, 0, 2))

    bridgeT = np.ascontiguousarray(
        bridge_np.reshape(N, D).T)                          # [64, 16384]

    # dense weight matrices
    # d1: g[n, o*20 + i*5 + k] = sum_c bridge[n, c*4+i] * d1_w[c, o, k]
    d1_wn = np.asarray(d1_w, np.float32)
    Wd1 = np.zeros((D, 640), np.float32)
    for c_i in range(16):
        for i in range(4):
            for o in range(32):
                for kk in range(5):
                    Wd1[c_i * 4 + i, o * 20 + i * 5 + kk] = d1_wn[c_i, o, kk]
    d1_bias_full = np.repeat(np.asarray(d1_b, np.float32), 20)      # [640]
    Wd2 = _conv_W(np.asarray(d2_w, np.float32), 20)                 # [640, 120]
    d2_bias_full = np.repeat(np.asarray(d2_b, np.float32), 20)      # [120]

    Wae1 = _conv_W(np.asarray(ae1_w, np.float32), MAL)              # [640, 320]
    ae1_bias_full = np.repeat(np.asarray(ae1_b, np.float32), MAL)   # [320]
    Wae2 = _conv_W(np.asarray(ae2_w, np.float32), 5)                # [160, 80]
    ae2_bias_full = np.repeat(np.asarray(ae2_b, np.float32), 5)     # [80]
    Wae3 = np.asarray(ae3_w, np.float32)                            # [80, 32]
    Wad1 = np.asarray(ad1_w, np.float32)                            # [32, 256]
    Wad2 = np.asarray(ad2_w, np.float32)                            # [256, 7680]

    in_common = {
        "wd1": _bf16(Wd1),
        "d1b": d1_bias_full.reshape(5, 128).T.copy(),
        "wd2": _bf16(Wd2.reshape(5, 128, 120).transpose(1, 0, 2)),
        "wae1": _bf16(Wae1.reshape(5, 128, 320).transpose(1, 0, 2)),
        "ae1b": _bf16(ae1_bias_full.reshape(1, 320)),
        "wae2a": _bf16(Wae2[:128]),
        "wae2b": _bf16(Wae2[128:]),
        "ae2b": ae2_bias_full.reshape(80, 1).copy(),
        "wae3": _bf16(Wae3),
        "ae3b": np.asarray(ae3_b, np.float32).reshape(32, 1).copy(),
        "wad1": _bf16(Wad1),
        "ad1b": np.asarray(ad1_b, np.float32).reshape(2, 128).T.copy(),
        "wad2": _bf16(Wad2.reshape(2, 128, 7680).transpose(1, 0, 2)),
    }
    in_maps = []
    for c_i in range(NCORES):
        m = dict(in_common)
        m["bridgeT"] = _bf16(bridgeT[:, c_i * NW:(c_i + 1) * NW])
        m["afT"] = _bf16(afT_ch[:, :, c_i * a_pad:(c_i + 1) * a_pad])
        in_maps.append(m)

    # ---- compile + run the bass kernel ----
    key = (a_pad, a_chunk)
    if key not in _KERNEL_CACHE:
        _KERNEL_CACHE[key] = _build_bass(a_pad, a_chunk)
    nc = _KERNEL_CACHE[key]
    res = run_bass_kernel_spmd(nc, in_maps, core_ids=list(range(NCORES)))
    LAST_RESULTS.clear()
    LAST_RESULTS.append(res)

    imu_gen = np.concatenate(
        [res.results[i]["imu"].astype(np.float32) for i in range(NCORES)],
        axis=0)
    imu_gen = (imu_gen + d2_bias_full).reshape(BS, SEQ, C, L)
    atom_full = np.concatenate(
        [res.results[i]["atom"][:a_chunk].astype(np.float32)
         for i in range(NCORES)], axis=0)
    ad2_bn = np.asarray(ad2_b, np.float32).reshape(7680)
    atom_gen = (atom_full[:A] + ad2_bn).reshape(A, C, WTOT)

    bridge_resh = np.asarray(forcast_in).reshape(N, 16, 4)
    return (imu_gen, atom_gen, imu_atoms_mask, imu_atoms, bridge_resh,
            np.asarray(forcast_in), np.asarray(forcast), np.asarray(fmask),
            np.asarray(forcast_loss))
